# revision 1
# baseline (speedup 1.0000x reference)
"""DSTP-RNN (dual-stage two-phase attention RNN) Trainium2 Bass kernel.

Sharding: pure data-parallel over batch. B=1024 split across 8 NeuronCores,
128 batch rows per core = the 128 SBUF partitions. Weights replicated.

Per-core design:
  - LSTM state transposed: hT/cT are [H=128 part, B=128 free] carrying 2*h and
    2*c; weights consuming them are pre-scaled 0.5 host-side. This makes
    sigmoid available from the tanh table (sigmoid(z)=0.5+0.5*tanh(z/2))
    with zero per-step fixups. Only the exp/tanh activation-table set is
    ever loaded. h is carried in bf16; c stays fp32.
  - Attention tensors are [batch part, (feature, time) free] bf16 with the
    broadcast operand always inner-stride-1 so DVE runs in the 2x perf mode.
  - LSTM biases enter via hi/lo-split K=2 ones-row matmuls issued with the
    h-side gate matmuls right after weps, so the whole constant part of the
    gate pre-activation accumulates in PSUM while the attention block runs;
    only the x-side matmuls remain on the per-step critical tail.
  - fin is stored transposed [B, (h, t)] so the decoder context reduction
    (din) is an inner-stride-1 multiply + contiguous halving tree (2x mode).
  - The decoder attention is computed transposed [h, (t, b)]: the score
    contraction over h=128 runs on the PE as per-t matmuls (etan t-block
    stationary, vd as the 1-column moving operand) writing score columns
    straight into a [B, T] PSUM tile; softmax normalization is deferred into
    one per-partition scale of din; din is computed in two t-phases so the
    first 24 columns overlap the second half's tanh.
  - The Pool engine (otherwise idle) takes a tuned slice of each big
    per-step elementwise block plus the off-critical-path stores.
  - pre1/pre2 (step-invariant attention projections) fold their bias via a
    ones-row into the matmul and copy PSUM->SBUF in batched waves.
"""
import sys
import numpy as np

sys.path.insert(0, "/opt/trn_rl_repo")

import concourse.bass as bass
import concourse.mybir as mybir
from concourse.tile import TileContext

FP = mybir.dt.float32
BF = mybir.dt.bfloat16
AX = mybir.AxisListType
OP = mybir.AluOpType
AF = mybir.ActivationFunctionType

B, T, TD, H, NI = 1024, 48, 24, 128, 18
F1, F2, ND = 17, 129, 30
NCORES = 8
BL = B // NCORES
F2_CH = [(0,25),(25,25),(50,25),(75,25)]  # stage-2 DVE chunks
F2_POOL = [(100,14),(114,15)]  # stage-2 feature chunks on Pool
TD2 = 24                         # decoder score/din split point
TD_CH = [(0,12),(12,12),(24,12)]  # decoder argT time chunks on DVE
TD_POOL = (36, 12)               # decoder argT time chunk on Pool
DIN_CH = [(0, 56), (56, 56)]     # decoder din h-chunks on DVE
DIN_POOL = (112, 16)             # decoder din h-chunk on Pool
USE_TREE = True                  # halving trees vs single strided reduce


def _split_matmul_waits(nc):
    """Walrus allows one sync-wait slot per instruction. Hoist extra waits
    onto same-engine EventSemaphore instructions inserted just before."""
    f = nc.m.functions[0]
    new_blocks = []
    for blk in f.blocks:
        insts = blk.instructions
        out = []
        changed = False
        for ins in insts:
            si = ins.sync_info
            if (si is not None and len(si.on_wait) > 1
                    and type(ins).__name__ != "InstEventSemaphore"
                    and getattr(ins, "engine", None) is not None
                    and str(ins.engine) != "EngineType.Unassigned"):
                waits = list(si.on_wait)
                for k, wt in enumerate(waits[:-1]):
                    ev = mybir.InstEventSemaphore(name=f"{ins.name}-wt{k}")
                    ev.engine = ins.engine
                    ev.sync_info = mybir.SyncInfo(on_wait=[wt], on_update=[])
                    out.append(ev)
                ins.sync_info = mybir.SyncInfo(
                    on_wait=[waits[-1]], on_update=list(si.on_update))
                changed = True
            out.append(ins)
        if changed:
            nb = mybir.BasicBlock(name=blk.name, instructions=out)
            for attr in ("IsExit", "IsLoopEntry", "IsPredicated"):
                val = getattr(blk, attr)
                if val is not None:
                    setattr(nb, attr, val)
            new_blocks.append(nb)
        else:
            new_blocks.append(blk)
    f.blocks = new_blocks


INS = [
    ("xTT", [T + 1, F1 * BL], BF),  # xTT[t, f*BL+b] = x[b,t,f]; row T = ones
    ("xT", [F1, T * BL], BF),       # xT[f, t*BL+b] = x[b,t,f]
    ("lab", [BL, T], FP),
    ("we1T_h", [H, T], BF), ("we1T_c", [H, T], FP),
    ("ue1Ta", [T + 1, T], BF), ("ve1_rep", [BL, T], BF),
    ("wih1T", [F1, 4 * H], BF), ("whh1T", [H, 4 * H], BF),
    ("we2T_h", [H, T], BF), ("we2T_c", [H, T], FP),
    ("ue2Ta", [T + 1, T], BF), ("ve2_rep", [BL, T], BF),
    ("wih2T_h", [H, 4 * H], BF), ("wih2T_l", [1, 4 * H], BF),
    ("bias1r", [2, 4 * H], BF), ("bias2r", [2, 4 * H], BF),
    ("whh2T", [H, 4 * H], BF),
    ("udT", [H, H], BF), ("udbT", [H, 1], FP),
    ("wdT_h", [H, H], BF), ("wdT_c", [H, H], FP), ("vdcol", [H, 1], BF),
    ("wihdT", [H, 4 * H], BF), ("whhdT", [H, 4 * H], BF),
    ("biasd", [2, 4 * H], BF), ("ones2", [2, BL], BF),
    ("regWT", [H, 1], BF), ("regb_rep", [BL, 1], FP),
    ("identf", [128, 128], FP), ("identb", [128, 128], BF),
]


def build_bass(reps=1, split_waits=True):
    nc = bass.Bass()
    d = {name: nc.dram_tensor(name, list(shape), dt, kind="ExternalInput")
         for name, shape, dt in INS}
    out_d = nc.dram_tensor("out", [BL, TD], FP, kind="ExternalOutput")

    with TileContext(nc) as tc:
        with (
            tc.tile_pool(name="pw", bufs=1) as pw,
            tc.tile_pool(name="pst", bufs=1) as pst,
            tc.tile_pool(name="psm", bufs=3) as psm,
            tc.tile_pool(name="pio", bufs=1) as pio,
            tc.tile_pool(name="pps", bufs=2, space="PSUM") as ppsum,
            tc.tile_pool(name="ptr", bufs=2, space="PSUM") as ptr,
        ):
            w = {}
            for name, shape, dt in INS:
                if name in ("xTT", "xT"):
                    continue
                tile = pw.tile(list(shape), dt, name="w_" + name,
                               tag="w_" + name)
                nc.gpsimd.dma_start(out=tile[:, :], in_=d[name][:, :])
                w[name] = tile
            idf, idb = w["identf"], w["identb"]

            def new_state(pfx, hdt):
                hT = pst.tile([H, BL], hdt, name=pfx + "h0", tag=pfx + "h",
                              bufs=2)
                cT = pst.tile([H, BL], FP, name=pfx + "c0", tag=pfx + "c",
                              bufs=2)
                nc.vector.memset(hT[:, :], 0.0)
                nc.vector.memset(cT[:, :], 0.0)
                return hT, cT

            def lstm_gates_h(pfx, t, hTb, whhT, biasrow=None):
                """h-side gate matmuls (plus constant bias rows when they
                cannot ride an x-side ones-row), issued right after weps so
                the PE runs them while the attention block computes."""
                gps = ppsum.tile([H, 4 * H], FP, name=f"{pfx}g{t}",
                                 tag="gates")
                for g in range(4):
                    gs = slice(g * H, (g + 1) * H)
                    nc.tensor.matmul(gps[:, gs], whhT[:, gs], hTb[:, :],
                                     start=True, stop=False)
                    if biasrow is not None:
                        nc.tensor.matmul(gps[:, gs], biasrow[:, gs],
                                         w["ones2"][:, :], start=False,
                                         stop=False)
                return gps

            def lstm_gates_x(gps, xsT, wihT, xlabT=None, wlabT=None):
                """x-side gate matmuls; bias rides in via ones-rows of xsT
                (or xlabT) matched by bias rows appended to wihT (wlabT)."""
                for g in range(4):
                    gs = slice(g * H, (g + 1) * H)
                    if xlabT is None:
                        nc.tensor.matmul(gps[:, gs], wihT[:, gs], xsT,
                                         start=False, stop=True)
                    else:
                        nc.tensor.matmul(gps[:, gs], wihT[:, gs], xsT,
                                         start=False, stop=False)
                        nc.tensor.matmul(gps[:, gs], wlabT[:, gs], xlabT,
                                         start=False, stop=True)
                return gps

            def lstm_update(pfx, t, gps, cT, hdt):
                """i/f/o share one merged tanh(0.5*.) activation; bias was
                accumulated into gps by the gate matmuls."""
                Ab = psm.tile([H, 3 * BL], FP, name=f"{pfx}ab{t}", tag="actA")
                C = psm.tile([H, BL], FP, name=f"{pfx}cg{t}", tag="actC")
                nc.scalar.activation(Ab[:, :], gps[:, 0:3 * H], AF.Tanh,
                                     scale=0.5)
                nc.scalar.activation(C[:, :], gps[:, 3 * H:4 * H], AF.Tanh)
                Bv, A, D = (Ab[:, 0:BL], Ab[:, BL:2 * BL], Ab[:, 2 * BL:])
                u = psm.tile([H, BL], FP, name=f"{pfx}u{t}", tag="lu")
                nc.vector.scalar_tensor_tensor(u[:, :], A, 1.0,
                                               cT[:, :], op0=OP.add,
                                               op1=OP.mult)
                v = psm.tile([H, BL], FP, name=f"{pfx}v{t}", tag="lv")
                nc.vector.scalar_tensor_tensor(v[:, :], Bv, 1.0,
                                               C[:, :], op0=OP.add,
                                               op1=OP.mult)
                cT2 = pst.tile([H, BL], FP, name=f"{pfx}c{t}", tag=pfx + "c",
                               bufs=2)
                nc.vector.scalar_tensor_tensor(cT2[:, :], u[:, :], 0.5,
                                               v[:, :], op0=OP.mult,
                                               op1=OP.add)
                Tc = psm.tile([H, BL], FP, name=f"{pfx}T{t}", tag="lTc")
                nc.scalar.activation(Tc[:, :], cT2[:, :], AF.Tanh, scale=0.5)
                hT2 = pst.tile([H, BL], hdt, name=f"{pfx}h{t}",
                               tag=pfx + "h", bufs=2)
                nc.vector.scalar_tensor_tensor(hT2[:, :], D, 1.0,
                                               Tc[:, :], op0=OP.add,
                                               op1=OP.mult)
                return hT2, cT2

            def softmax(pfx, t, score, F):
                """exp-normalize (no max shift); returns bf16 sm [BL,F]."""
                exps = psm.tile([BL, F], FP, name=f"{pfx}e{t}", tag="exps")
                sume = psm.tile([BL, 1], FP, name=f"{pfx}se{t}", tag="sume")
                nc.scalar.activation(exps[:, :], score[:, :], AF.Exp,
                                     accum_out=sume[:, :])
                rs = psm.tile([BL, 1], FP, name=f"{pfx}rs{t}", tag="rs")
                nc.vector.reciprocal(rs[:, :], sume[:, :])
                sm = psm.tile([BL, F], BF, name=f"{pfx}sm{t}", tag="sm")
                nc.vector.tensor_scalar_mul(sm[:, :], exps[:, :], rs[:, :])
                return sm

            def sum_inner(eng, pool, pfx, t, ci, t2, fn, score_sl, S=T):
                """sum over innermost S of bf16 [BL, fn, S] into score_sl.
                GPSIMD cannot reduce along free dims, so Pool chunks tree
                down and hand the last small reduce to DVE."""
                if not USE_TREE:
                    nc.vector.reduce_sum(
                        score_sl,
                        t2[:, :].rearrange("p (f s) -> p f s", s=S),
                        axis=AX.X)
                    return
                cur, size = t2, S
                lvls = []
                while size > 6:
                    size //= 2
                    lvls.append(size)
                for lvl, ns in enumerate(lvls):
                    nxt = pool.tile([BL, fn * ns], BF,
                                    name=f"{pfx}r{t}_{ci}_{lvl}",
                                    tag=f"{pfx}tr{ci}_{lvl}", bufs=1)
                    cv = cur[:, :].rearrange("p (f s) -> p f s", s=ns * 2)
                    eng.tensor_add(
                        nxt[:, :].rearrange("p (f s) -> p f s", s=ns),
                        cv[:, :, 0:ns], cv[:, :, ns:2 * ns])
                    cur = nxt
                nc.vector.reduce_sum(
                    score_sl,
                    cur[:, :].rearrange("p (f s) -> p f s", s=lvls[-1]),
                    axis=AX.X)

            for _rep in range(reps):
                # =================== STAGE 1 ===================
                mid_sb = pio.tile([BL, T * F2], BF, name="mid_sb",
                                  tag="mid_sb")
                fin_sb = pio.tile([BL, H * T], BF, name="fin_sb",
                                  tag="fin_sb")   # [b, (h, t)] h-major
                out_sb = pio.tile([BL, ND], FP, name="out_sb", tag="out_sb")
                pre2 = pio.tile([BL, F2 * T], BF, name="pre2", tag="pre2")
                midT = pio.tile([H, T * BL], BF, name="midT", tag="midT")
                udT = pio.tile([H, T * BL], BF, name="udT", tag="udT")
                with tc.tile_pool(name="ps1", bufs=1) as ps1:
                    xT_sb = ps1.tile([F1, T * BL], BF, name="xT_sb",
                                     tag="xT_sb")
                    nc.gpsimd.dma_start(out=xT_sb[:, :], in_=d["xT"][:, :])
                    pre1 = ps1.tile([BL, F1 * T], BF, name="pre1",
                                    tag="pre1")

                    with tc.tile_pool(name="pxtt", bufs=1) as pxtt:
                        xTT_sb = pxtt.tile([T + 1, F1 * BL], BF,
                                           name="xTT_sb", tag="xTT_sb")
                        nc.gpsimd.dma_start(out=xTT_sb[:, :],
                                            in_=d["xTT"][:, :])
                        for f0 in range(0, F1, 4):
                            fn = min(4, F1 - f0)
                            pp = ppsum.tile([BL, 4 * T], FP,
                                            name=f"pp1_{f0}", tag="mm_pre")
                            for j in range(fn):
                                nc.tensor.matmul(
                                    pp[:, j * T:(j + 1) * T],
                                    xTT_sb[:, (f0 + j) * BL:(f0 + j + 1) * BL],
                                    w["ue1Ta"][:, :], start=True, stop=True)
                            nc.vector.tensor_copy(
                                pre1[:, f0 * T:(f0 + fn) * T],
                                pp[:, 0:fn * T])

                    nc.vector.tensor_copy(
                        mid_sb[:, :].rearrange("p (t f) -> p t f",
                                               f=F2)[:, :, H],
                        w["lab"][:, :])
                    hT, cT = new_state("s1", BF)
                    for t in range(T):
                        weps = ppsum.tile([BL, T], FP, name=f"we1_{t}",
                                          tag="mm_small")
                        nc.tensor.matmul(weps[:, :], cT[:, :],
                                         w["we1T_c"][:, :], start=True,
                                         stop=False)
                        nc.tensor.matmul(weps[:, :], hT[:, :],
                                         w["we1T_h"][:, :], start=False,
                                         stop=True)
                        gps = lstm_gates_h("s1", t, hT, w["whh1T"], biasrow=w["bias1r"])
                        we_sb = psm.tile([BL, T], BF, name=f"wes1_{t}",
                                         tag="we_sb")
                        nc.vector.tensor_copy(we_sb[:, :], weps[:, :])
                        score = psm.tile([BL, F1], FP, name=f"sc1_{t}",
                                         tag="sc")
                        s1ch = [(0, 6), (6, 6), (12, 5)]
                        s1eargs, s1etans = [], []
                        for ci, (f0, fn) in enumerate(s1ch):
                            earg = ps1.tile([BL, fn * T], BF,
                                            name=f"ea1_{t}_{ci}",
                                            tag=f"big1A{ci}", bufs=2)
                            nc.vector.tensor_add(
                                earg[:, :].rearrange("p (f s) -> p f s",
                                                     s=T),
                                pre1[:, f0 * T:(f0 + fn) * T].rearrange(
                                    "p (f s) -> p f s", s=T),
                                we_sb[:, :].unsqueeze(1).broadcast_to(
                                    [BL, fn, T]))
                            s1eargs.append(earg)
                        for ci, (f0, fn) in enumerate(s1ch):
                            etan = ps1.tile([BL, fn * T], BF,
                                            name=f"et1_{t}_{ci}",
                                            tag=f"big1B{ci}", bufs=2)
                            nc.scalar.activation(etan[:, :],
                                                 s1eargs[ci][:, :], AF.Tanh)
                            s1etans.append(etan)
                        for ci, (f0, fn) in enumerate(s1ch):
                            t2 = ps1.tile([BL, fn * T], BF,
                                          name=f"t21_{t}_{ci}",
                                          tag=f"big1A{ci}", bufs=2)
                            nc.vector.tensor_mul(
                                t2[:, :].rearrange("p (f s) -> p f s", s=T),
                                s1etans[ci][:, :].rearrange(
                                    "p (f s) -> p f s", s=T),
                                w["ve1_rep"][:, :].unsqueeze(1).broadcast_to(
                                    [BL, fn, T]))
                            nc.vector.reduce_sum(
                                score[:, f0:f0 + fn],
                                t2[:, :].rearrange("p (f s) -> p f s", s=T),
                                axis=AX.X)
                        sm = softmax("s1", t, score, F1)
                        smT = ptr.tile([F1, BL], BF, name=f"smT1_{t}",
                                       tag="tr")
                        nc.tensor.transpose(smT[:, :], sm[:, :], idb[:, :])
                        xsT = psm.tile([F1, BL], BF, name=f"xsT1_{t}",
                                       tag="xsT")
                        nc.vector.tensor_mul(xsT[:, :],
                                             xT_sb[:,
                                                   t * BL:(t + 1) * BL],
                                             smT[:, :])
                        lstm_gates_x(gps, xsT, w["wih1T"])
                        hT, cT = lstm_update("s1", t, gps, cT, BF)
                        nc.gpsimd.tensor_scalar_mul(
                            midT[:, t * BL:(t + 1) * BL], hT[:, :], 0.5)
                        hbt = ptr.tile([BL, H], BF, name=f"hbt_{t}",
                                       tag="tr")
                        nc.tensor.transpose(hbt[:, :], hT[:, :], idb[:, :])
                        nc.vector.tensor_scalar_mul(
                            mid_sb[:, t * F2:t * F2 + H], hbt[:, :], 0.5)

                    # ---- pre2 build: waves of 8 f ----
                    mid3 = mid_sb[:, :].rearrange("p (t f) -> p t f", f=F2)
                    mscs = []
                    for k in range(4):
                        m = ps1.tile([T + 1, BL], BF, name=f"msc{k}",
                                     tag=f"msc{k}")
                        nc.vector.memset(m[32:T + 1, :], 1.0)
                        mscs.append(m)
                    cp_eng = [nc.vector, nc.scalar]
                    for f0 in range(0, F2, 8):
                        fn = min(8, F2 - f0)
                        pp = ppsum.tile([BL, 8 * T], FP, name=f"pp2_{f0}",
                                        tag="mm_pre")
                        mtps = []
                        for j in range(fn):
                            mtp = ptr.tile([T, BL], BF,
                                           name=f"mtp_{f0 + j}", tag="tr")
                            nc.tensor.transpose(mtp[:, :],
                                                mid3[:, :, f0 + j],
                                                idb[:, :])
                            mtps.append(mtp)
                        for j in range(fn):
                            eng = cp_eng[(f0 + j) % 2]
                            msc = mscs[(f0 + j) % 4]
                            if eng is nc.scalar:
                                eng.copy(msc[0:T, :], mtps[j][:, :])
                            else:
                                eng.tensor_copy(msc[0:T, :], mtps[j][:, :])
                            nc.tensor.matmul(pp[:, j * T:(j + 1) * T],
                                             msc[:, :], w["ue2Ta"][:, :],
                                             start=True, stop=True)
                        nc.vector.tensor_copy(pre2[:, f0 * T:(f0 + fn) * T],
                                              pp[:, 0:fn * T])

                # =================== STAGE 2 ===================
                with tc.tile_pool(name="ps2", bufs=1) as ps2:
                    hT, cT = new_state("s2", BF)
                    for t in range(T):
                        weps = ppsum.tile([BL, T], FP, name=f"we2_{t}",
                                          tag="mm_small")
                        nc.tensor.matmul(weps[:, :], cT[:, :],
                                         w["we2T_c"][:, :], start=True,
                                         stop=False)
                        nc.tensor.matmul(weps[:, :], hT[:, :],
                                         w["we2T_h"][:, :], start=False,
                                         stop=True)
                        gps = lstm_gates_h("s2", t, hT, w["whh2T"], biasrow=w["bias2r"])
                        we_sb = psm.tile([BL, T], BF, name=f"wes2_{t}",
                                         tag="we_sb")
                        nc.vector.tensor_copy(we_sb[:, :], weps[:, :])
                        score = psm.tile([BL, F2], FP, name=f"sc2_{t}",
                                         tag="sc")
                        chunks = ([(*F2_POOL[0], nc.gpsimd)]
                                  + [(f0, fn, nc.vector) for f0, fn in F2_CH]
                                  + [(f0, fn, nc.gpsimd)
                                     for f0, fn in F2_POOL[1:]])
                        eargs, etans = [], []
                        for ci, (f0, fn, eng) in enumerate(chunks):
                            earg = ps2.tile([BL, fn * T], BF,
                                            name=f"ea2_{t}_{ci}",
                                            tag=f"big2A{ci}", bufs=2)
                            eng.tensor_add(
                                earg[:, :].rearrange("p (f s) -> p f s",
                                                     s=T),
                                pre2[:, f0 * T:(f0 + fn) * T].rearrange(
                                    "p (f s) -> p f s", s=T),
                                we_sb[:, :].unsqueeze(1).broadcast_to(
                                    [BL, fn, T]))
                            eargs.append(earg)
                        for ci, (f0, fn, eng) in enumerate(chunks):
                            etan = ps2.tile([BL, fn * T], BF,
                                            name=f"et2_{t}_{ci}",
                                            tag=f"big2B{ci}", bufs=2)
                            nc.scalar.activation(etan[:, :],
                                                 eargs[ci][:, :], AF.Tanh)
                            etans.append(etan)
                        for ci, (f0, fn, eng) in enumerate(chunks):
                            t2 = ps2.tile([BL, fn * T], BF,
                                          name=f"t22_{t}_{ci}",
                                          tag=f"big2A{ci}", bufs=2)
                            eng.tensor_mul(
                                t2[:, :].rearrange("p (f s) -> p f s", s=T),
                                etans[ci][:, :].rearrange(
                                    "p (f s) -> p f s", s=T),
                                w["ve2_rep"][:, :].unsqueeze(1).broadcast_to(
                                    [BL, fn, T]))
                            sum_inner(eng, ps2, "s2", t, ci, t2, fn,
                                      score[:, f0:f0 + fn])
                        sm = softmax("s2", t, score, F2)
                        smT = ptr.tile([H, BL], BF, name=f"smT2_{t}",
                                       tag="tr")
                        nc.tensor.transpose(smT[:, :], sm[:, 0:H],
                                            idb[:, :])
                        xsT = psm.tile([H, BL], BF, name=f"xsT2_{t}",
                                       tag="xsT")
                        nc.vector.tensor_mul(xsT[:, :],
                                             midT[:, t * BL:(t + 1) * BL],
                                             smT[:, :])
                        xlab = psm.tile([BL, 1], FP, name=f"xl_{t}",
                                        tag="s2xl")
                        nc.vector.tensor_mul(xlab[:, :],
                                             w["lab"][:, t:t + 1],
                                             sm[:, H:H + 1])
                        xlT_ps = ptr.tile([1, BL], FP, name=f"xlT_{t}",
                                          tag="tr")
                        nc.tensor.transpose(xlT_ps[:, :], xlab[:, :],
                                            idf[:, :])
                        xlabT = psm.tile([1, BL], BF, name=f"xls_{t}",
                                         tag="s2xls")
                        nc.vector.tensor_copy(xlabT[:, :], xlT_ps[:, :])
                        lstm_gates_x(gps, xsT, w["wih2T_h"], xlabT=xlabT,
                                     wlabT=w["wih2T_l"])
                        hT, cT = lstm_update("s2", t, gps, cT, BF)
                        hbt = ptr.tile([BL, H], BF, name=f"hbt2_{t}",
                                       tag="tr")
                        nc.tensor.transpose(hbt[:, :], hT[:, :], idb[:, :])
                        nc.scalar.mul(
                            fin_sb[:, :].rearrange("p (h t) -> p h t",
                                                   t=T)[:, :, t],
                            hbt[:, :], 0.5)
                        up = ppsum.tile([H, BL], FP, name=f"udp_{t}",
                                        tag="mm_small")
                        nc.tensor.matmul(up[:, :], w["udT"][:, :], hT[:, :],
                                         start=True, stop=True)
                        nc.scalar.activation(udT[:, t * BL:(t + 1) * BL],
                                             up[:, :], AF.Identity,
                                             bias=w["udbT"][:, 0:1])

                # =================== DECODER ===================
                # Attention computed transposed [h, (t, b)]: the score
                # contraction over h=128 then runs on the PE (48 tiny
                # matmuls with the constant vd stationary) instead of a
                # DVE multiply + tree, and softmax normalization is
                # deferred into a per-partition scale of din.
                with tc.tile_pool(name="pdec", bufs=1) as pdec:
                    fin3 = fin_sb[:, :].rearrange("p (h s) -> p h s", s=T)
                    hT, cT = new_state("sd", BF)
                    hTb = hT
                    for t in range(ND):
                        wdT_ps = ppsum.tile([H, BL], FP, name=f"wd_{t}",
                                            tag="mm_small")
                        nc.tensor.matmul(wdT_ps[:, :], w["wdT_c"][:, :],
                                         cT[:, :], start=True, stop=False)
                        nc.tensor.matmul(wdT_ps[:, :], w["wdT_h"][:, :],
                                         hTb[:, :], start=False, stop=True)
                        gps = lstm_gates_h("sd", t, hTb, w["whhdT"],
                                           biasrow=w["biasd"])
                        wdTs = psm.tile([H, BL], BF, name=f"wds_{t}",
                                        tag="we_sb")
                        nc.vector.tensor_copy(wdTs[:, :], wdT_ps[:, :])
                        achunks = ([(t0, tn, nc.vector) for t0, tn in TD_CH]
                                   + [(*TD_POOL, nc.gpsimd)])
                        eargs = []
                        for ci, (t0, tn, eng) in enumerate(achunks):
                            argT = pdec.tile([H, tn * BL], BF,
                                             name=f"ead_{t}_{ci}",
                                             tag=f"bigdA{ci}", bufs=2)
                            eng.tensor_add(
                                argT[:, :].rearrange("p (s b) -> p s b",
                                                     b=BL),
                                udT[:, t0 * BL:(t0 + tn) * BL].rearrange(
                                    "p (s b) -> p s b", b=BL),
                                wdTs[:, :].unsqueeze(1).broadcast_to(
                                    [H, tn, BL]))
                            eargs.append(argT)
                        scps = ppsum.tile([BL, T], FP, name=f"scT_{t}",
                                          tag="mm_small")

                        def etan_scores(ci):
                            t0, tn, eng = achunks[ci]
                            etan = pdec.tile([H, tn * BL], BF,
                                             name=f"etd_{t}_{ci}",
                                             tag=f"bigdB{ci}", bufs=2)
                            nc.scalar.activation(etan[:, :],
                                                 eargs[ci][:, :], AF.Tanh)
                            for j in range(tn):
                                nc.tensor.matmul(
                                    scps[:, t0 + j:t0 + j + 1],
                                    etan[:, j * BL:(j + 1) * BL],
                                    w["vdcol"][:, :],
                                    start=True, stop=True)

                        def din_half(ph, t0, tn, dout):
                            expv = psm.tile([BL, tn], BF,
                                            name=f"exb{ph}_{t}",
                                            tag=f"expsb{ph}")
                            Zp = psm.tile([BL, 1], FP, name=f"Z{ph}_{t}",
                                          tag=f"Zs{ph}")
                            nc.scalar.activation(expv[:, :],
                                                 scps[:, t0:t0 + tn],
                                                 AF.Exp,
                                                 accum_out=Zp[:, :])
                            dchunks = ([(h0, hn, nc.vector)
                                        for h0, hn in DIN_CH]
                                       + [(*DIN_POOL, nc.gpsimd)])
                            for ci, (h0, hn, eng) in enumerate(dchunks):
                                dmul = pdec.tile([BL, hn * tn], BF,
                                                 name=f"dm{ph}_{t}_{ci}",
                                                 tag=f"bigd{ph}D{ci}",
                                                 bufs=1)
                                eng.tensor_mul(
                                    dmul[:, :].rearrange(
                                        "p (h s) -> p h s", s=tn),
                                    fin3[:, h0:h0 + hn, t0:t0 + tn],
                                    expv[:, :].unsqueeze(1).broadcast_to(
                                        [BL, hn, tn]))
                                sum_inner(eng, pdec, f"sd{ph}", t, ci,
                                          dmul, hn, dout[:, h0:h0 + hn],
                                          S=tn)
                            return Zp

                        etan_scores(0)
                        etan_scores(1)
                        dinA = psm.tile([BL, H], FP, name=f"dinA_{t}",
                                        tag="sddinA")
                        ZA = din_half("A", 0, TD2, dinA)
                        etan_scores(2)
                        etan_scores(3)
                        dinB = psm.tile([BL, H], FP, name=f"dinB_{t}",
                                        tag="sddinB")
                        ZB = din_half("B", TD2, T - TD2, dinB)
                        Zs = psm.tile([BL, 1], FP, name=f"Zs_{t}", tag="Zs")
                        nc.vector.tensor_add(Zs[:, :], ZA[:, :], ZB[:, :])
                        rsd = psm.tile([BL, 1], FP, name=f"rsd_{t}",
                                       tag="rs")
                        nc.vector.reciprocal(rsd[:, :], Zs[:, :])
                        din = psm.tile([BL, H], FP, name=f"din_{t}",
                                       tag="sddin")
                        nc.vector.tensor_add(din[:, :], dinA[:, :],
                                             dinB[:, :])
                        dinn = psm.tile([BL, H], FP, name=f"dinn_{t}",
                                        tag="sddinN")
                        nc.vector.tensor_scalar_mul(dinn[:, :], din[:, :],
                                                    rsd[:, :])
                        dinT_ps = ptr.tile([H, BL], FP, name=f"dTp_{t}",
                                           tag="tr")
                        nc.tensor.transpose(dinT_ps[:, :], dinn[:, :],
                                            idf[:, :])
                        dinT = psm.tile([H, BL], BF, name=f"dT_{t}",
                                        tag="sddinT")
                        nc.vector.tensor_copy(dinT[:, :], dinT_ps[:, :])
                        lstm_gates_x(gps, dinT, w["wihdT"])
                        hT, cT = lstm_update("sd", t, gps, cT, BF)
                        hTb = hT
                        op = ppsum.tile([BL, 1], FP, name=f"op_{t}",
                                        tag="mm_small")
                        nc.tensor.matmul(op[:, :], hT[:, :],
                                         w["regWT"][:, :], start=True,
                                         stop=True)
                        nc.vector.tensor_copy(out_sb[:, t:t + 1], op[:, :])

                    outf = pdec.tile([BL, TD], FP, name="outf", tag="outf")
                    nc.vector.tensor_scalar_add(outf[:, :], out_sb[:, 6:ND],
                                                w["regb_rep"][:, :])
                    nc.gpsimd.dma_start(out=out_d[:, :], in_=outf[:, :])

    if split_waits:
        _split_matmul_waits(nc)
    return nc


# ---------------- host-side prep ----------------
def _prep_weights(i):
    f32 = lambda a: np.ascontiguousarray(a, np.float32)
    try:
        import ml_dtypes
        bf16 = lambda a: np.ascontiguousarray(
            np.asarray(a, np.float32).astype(ml_dtypes.bfloat16))
    except ImportError:
        import jax.numpy as jnp
        bf16 = lambda a: np.ascontiguousarray(
            np.asarray(jnp.asarray(a, jnp.bfloat16)))
    # gate order [i, f, o, g]: i/f/o share the tanh(0.5*(.)) activation
    gperm = np.concatenate([np.arange(0, H), np.arange(H, 2 * H),
                            np.arange(3 * H, 4 * H), np.arange(2 * H, 3 * H)])

    def bias_hilo(b):
        bp = np.asarray(b, np.float32)[gperm]
        hi = np.asarray(bf16(bp), np.float32)
        return bp, hi, bp - hi

    w = {}
    w["we1T_h"] = bf16(0.5 * i["We1_W"][:, :H].T)
    w["we1T_c"] = f32(0.5 * i["We1_W"][:, H:].T)
    w["ue1Ta"] = bf16(np.concatenate(
        [i["Ue1_W"].T, i["Ue1_b"][None, :]], 0))
    w["ve1_rep"] = bf16(np.tile(i["Ve1_W"][0][None, :], (BL, 1)))
    b1, b1hi, b1lo = bias_hilo(i["e1_bih"] + i["e1_bhh"])
    w["wih1T"] = bf16(i["e1_Wih"].T[:, gperm])
    w["bias1r"] = bf16(np.stack([b1hi, b1lo], 0))
    w["whh1T"] = bf16(0.5 * i["e1_Whh"].T[:, gperm])
    w["we2T_h"] = bf16(0.5 * i["We2_W"][:, :H].T)
    w["we2T_c"] = f32(0.5 * i["We2_W"][:, H:].T)
    w["ue2Ta"] = bf16(np.concatenate(
        [i["Ue2_W"].T, i["Ue2_b"][None, :]], 0))
    w["ve2_rep"] = bf16(np.tile(i["Ve2_W"][0][None, :], (BL, 1)))
    b2, b2hi, b2lo = bias_hilo(i["e2_bih"] + i["e2_bhh"])
    w["wih2T_h"] = bf16(i["e2_Wih"][:, :H].T[:, gperm])
    w["wih2T_l"] = bf16(i["e2_Wih"][:, H:].T[:, gperm])
    w["bias2r"] = bf16(np.stack([b2hi, b2lo], 0))
    w["whh2T"] = bf16(0.5 * i["e2_Whh"].T[:, gperm])
    w["udT"] = bf16(0.5 * i["Ud_W"].T)
    w["udbT"] = f32(i["Ud_b"][:, None])
    w["wdT_h"] = bf16(0.5 * i["Wd_W"][:, :H].T)
    w["wdT_c"] = f32(0.5 * i["Wd_W"][:, H:].T)
    w["vdcol"] = bf16(i["Vd_W"][0][:, None])
    w["wihdT"] = bf16(i["d_Wih"].T[:, gperm])
    w["whhdT"] = bf16(0.5 * i["d_Whh"].T[:, gperm])
    bd, _, _ = bias_hilo(i["d_bih"] + i["d_bhh"])
    _hi = np.asarray(bf16(bd), np.float32)
    w["biasd"] = np.concatenate([bf16(bd)[None, :], bf16(bd - _hi)[None, :]],
                                0)
    w["ones2"] = bf16(np.ones((2, BL)))
    w["regWT"] = bf16(0.5 * i["reg_W"].T)
    w["regb_rep"] = f32(np.tile(i["reg_b"][None, :], (BL, 1)))
    w["identf"] = np.eye(128, dtype=np.float32)
    w["identb"] = bf16(np.eye(128))
    w["_bf16"] = bf16
    return w


def prep_in_maps(inputs):
    i = {k: np.asarray(v) for k, v in inputs.items()}
    wshared = _prep_weights(i)
    bf16 = wshared.pop("_bf16")
    maps = []
    for c in range(NCORES):
        sl = slice(c * BL, (c + 1) * BL)
        x = np.ascontiguousarray(i["input_p_q"][sl][:, :, 1:], np.float32)
        lab = np.ascontiguousarray(i["label_p"][sl], np.float32)
        m = dict(wshared)
        xTT = x.transpose(1, 2, 0).reshape(T, F1 * BL)
        m["xTT"] = bf16(np.concatenate(
            [xTT, np.ones((1, F1 * BL), np.float32)], 0))
        m["xT"] = bf16(x.transpose(2, 1, 0).reshape(F1, T * BL))
        m["lab"] = lab
        maps.append(m)
    return maps


_CACHE = {}


def kernel(**inputs):
    from concourse.bass_utils import run_bass_kernel_spmd
    if "nc" not in _CACHE:
        _CACHE["nc"] = build_bass()
    nc = _CACHE["nc"]
    in_maps = prep_in_maps(inputs)
    res = run_bass_kernel_spmd(nc, in_maps, list(range(NCORES)))
    out = np.concatenate([res.results[c]["out"] for c in range(NCORES)], 0)
    return out.astype(np.float32)


if __name__ == "__main__":
    nc = build_bass()
    print("built ok")



# revision 22
# speedup vs baseline: 1.0439x; 1.0439x over previous
"""DSTP-RNN (dual-stage two-phase attention RNN) Trainium2 Bass kernel.

Sharding: pure data-parallel over batch. B=1024 split across 8 NeuronCores,
128 batch rows per core = the 128 SBUF partitions. Weights replicated.

Per-core design:
  - LSTM state transposed: hT/cT are [H=128 part, B=128 free] carrying 2*h and
    2*c; weights consuming them are pre-scaled 0.5 host-side. This makes
    sigmoid available from the tanh table (sigmoid(z)=0.5+0.5*tanh(z/2))
    with zero per-step fixups. Only the exp/tanh activation-table set is
    ever loaded. h is carried in bf16; c stays fp32.
  - Attention tensors are [batch part, (feature, time) free] bf16 with the
    broadcast operand always inner-stride-1 so DVE runs in the 2x perf mode.
  - LSTM biases enter via hi/lo-split K=2 ones-row matmuls issued with the
    h-side gate matmuls right after weps, so the whole constant part of the
    gate pre-activation accumulates in PSUM while the attention block runs;
    only the x-side matmuls remain on the per-step critical tail.
  - fin is stored transposed [B, (h, t)] so the decoder context reduction
    (din) is an inner-stride-1 multiply + contiguous halving tree (2x mode).
  - The decoder attention is computed transposed [h, (t, b)]: the score
    contraction over h=128 runs on the PE as per-t matmuls (etan t-block
    stationary, vd as the 1-column moving operand) writing score columns
    straight into a [B, T] PSUM tile; softmax normalization is deferred into
    one per-partition scale of din; din is computed in two t-phases so the
    first 24 columns overlap the second half's tanh.
  - The Pool engine (otherwise idle) takes a tuned slice of each big
    per-step elementwise block plus the off-critical-path stores.
  - pre1/pre2 (step-invariant attention projections) fold their bias via a
    ones-row into the matmul and copy PSUM->SBUF in batched waves.
"""
import sys
import numpy as np

sys.path.insert(0, "/opt/trn_rl_repo")

import concourse.bass as bass
import concourse.mybir as mybir
from concourse.tile import TileContext

FP = mybir.dt.float32
BF = mybir.dt.bfloat16
AX = mybir.AxisListType
OP = mybir.AluOpType
AF = mybir.ActivationFunctionType

B, T, TD, H, NI = 1024, 48, 24, 128, 18
F1, F2, ND = 17, 129, 30
NCORES = 8
BL = B // NCORES
# stage-2 chunk table: (f0, fn, ea_kind, scan_kind); kinds: 'P' Pool,
# 'V' DVE. ea = broadcast add; scan = Horner tensor_tensor_scan that fuses
# the ve-weighted time reduction (tau axis is pre-permuted ascending |ve|
# host-side so the scan ratios are <= 1).
S2_CH = [
    (26, 19, "V", "V"),
    (0, 10, "P", "P"),
    (10, 16, "P", "V"),
    (45, 19, "V", "V"),
    (64, 19, "V", "V"),
    (83, 19, "V", "V"),
    (102, 17, "P", "V"),
    (119, 10, "P", "P"),
]
S2_SCAN_LO, S2_SCAN_HI = 10, 119  # contiguous f-range covered by scans
TD2 = 24                         # decoder score/din split point
TD_CH = [(0,13),(13,13),(26,14)]  # decoder argT time chunks on DVE
TD_POOL = (40, 8)                # decoder argT time chunk on Pool
DIN_CH = [(0, 52), (52, 52)]     # decoder din h-chunks on DVE
DIN_POOL = (104, 24)             # decoder din h-chunk on Pool
USE_TREE = True                  # halving trees vs single strided reduce


def _split_matmul_waits(nc):
    """Walrus allows one sync-wait slot per instruction. Hoist extra waits
    onto same-engine EventSemaphore instructions inserted just before."""
    f = nc.m.functions[0]
    new_blocks = []
    for blk in f.blocks:
        insts = blk.instructions
        out = []
        changed = False
        for ins in insts:
            si = ins.sync_info
            if (si is not None and len(si.on_wait) > 1
                    and type(ins).__name__ != "InstEventSemaphore"
                    and getattr(ins, "engine", None) is not None
                    and str(ins.engine) != "EngineType.Unassigned"):
                waits = list(si.on_wait)
                for k, wt in enumerate(waits[:-1]):
                    ev = mybir.InstEventSemaphore(name=f"{ins.name}-wt{k}")
                    ev.engine = ins.engine
                    ev.sync_info = mybir.SyncInfo(on_wait=[wt], on_update=[])
                    out.append(ev)
                ins.sync_info = mybir.SyncInfo(
                    on_wait=[waits[-1]], on_update=list(si.on_update))
                changed = True
            out.append(ins)
        if changed:
            nb = mybir.BasicBlock(name=blk.name, instructions=out)
            for attr in ("IsExit", "IsLoopEntry", "IsPredicated"):
                val = getattr(blk, attr)
                if val is not None:
                    setattr(nb, attr, val)
            new_blocks.append(nb)
        else:
            new_blocks.append(blk)
    f.blocks = new_blocks


INS = [
    ("xTT", [T + 1, F1 * BL], BF),  # xTT[t, f*BL+b] = x[b,t,f]; row T = ones
    ("xT", [F1, T * BL], BF),       # xT[f, t*BL+b] = x[b,t,f]
    ("lab", [BL, T], FP),
    ("we1T_h", [H, T], BF), ("we1T_c", [H, T], FP),
    ("ue1Ta", [T + 1, T], BF), ("r1full", [BL, F1 * T], FP),
    ("wih1T", [F1, 4 * H], BF), ("whh1T", [H, 4 * H], BF),
    ("we2T_h", [H, T], BF), ("we2T_c", [H, T], FP),
    ("ue2Ta", [T + 1, T], BF), ("r2full", [BL, F2 * T], FP),
    ("ve2_rep", [BL, T], BF),
    ("wih2T_h", [H, 4 * H], BF), ("wih2T_l", [1, 4 * H], BF),
    ("bias1r", [2, 4 * H], BF), ("bias2r", [2, 4 * H], BF),
    ("whh2T", [H, 4 * H], BF),
    ("udT", [H, H], BF), ("udbT", [H, 1], FP),
    ("wdT_h", [H, H], BF), ("wdT_c", [H, H], FP), ("vdcol", [H, 1], BF),
    ("wihdT", [H, 4 * H], BF), ("whhdT", [H, 4 * H], BF),
    ("biasd", [2, 4 * H], BF), ("ones2", [2, BL], BF),
    ("regWT", [H, 1], BF), ("regb_rep", [BL, 1], FP),
    ("vs1", [BL, 1], FP), ("vs2", [BL, 1], FP),
    ("identf", [128, 128], FP), ("identb", [128, 128], BF),
]


S2_CFG = None  # optional override of S2_CH for sweeps


def build_bass(reps=1, split_waits=True):
    s2_ch = S2_CFG if S2_CFG is not None else S2_CH
    nc = bass.Bass()
    d = {name: nc.dram_tensor(name, list(shape), dt, kind="ExternalInput")
         for name, shape, dt in INS}
    out_d = nc.dram_tensor("out", [BL, TD], FP, kind="ExternalOutput")

    with TileContext(nc) as tc:
        with (
            tc.tile_pool(name="pw", bufs=1) as pw,
            tc.tile_pool(name="pst", bufs=1) as pst,
            tc.tile_pool(name="psm", bufs=3) as psm,
            tc.tile_pool(name="pio", bufs=1) as pio,
            tc.tile_pool(name="pps", bufs=2, space="PSUM") as ppsum,
            tc.tile_pool(name="ptr", bufs=2, space="PSUM") as ptr,
        ):
            w = {}
            for name, shape, dt in INS:
                if name in ("xTT", "xT"):
                    continue
                tile = pw.tile(list(shape), dt, name="w_" + name,
                               tag="w_" + name)
                nc.gpsimd.dma_start(out=tile[:, :], in_=d[name][:, :])
                w[name] = tile
            idf, idb = w["identf"], w["identb"]

            def new_state(pfx, hdt):
                hT = pst.tile([H, BL], hdt, name=pfx + "h0", tag=pfx + "h",
                              bufs=2)
                cT = pst.tile([H, BL], FP, name=pfx + "c0", tag=pfx + "c",
                              bufs=2)
                nc.vector.memset(hT[:, :], 0.0)
                nc.vector.memset(cT[:, :], 0.0)
                return hT, cT

            def lstm_gates_h(pfx, t, hTb, whhT, biasrow=None):
                """h-side gate matmuls (plus constant bias rows when they
                cannot ride an x-side ones-row), issued right after weps so
                the PE runs them while the attention block computes."""
                gps = ppsum.tile([H, 4 * H], FP, name=f"{pfx}g{t}",
                                 tag="gates")
                for g in range(4):
                    gs = slice(g * H, (g + 1) * H)
                    nc.tensor.matmul(gps[:, gs], whhT[:, gs], hTb[:, :],
                                     start=True, stop=False)
                    if biasrow is not None:
                        nc.tensor.matmul(gps[:, gs], biasrow[:, gs],
                                         w["ones2"][:, :], start=False,
                                         stop=False)
                return gps

            def lstm_gates_x(gps, xsT, wihT, xlabT=None, wlabT=None):
                """x-side gate matmuls; bias rides in via ones-rows of xsT
                (or xlabT) matched by bias rows appended to wihT (wlabT)."""
                for g in range(4):
                    gs = slice(g * H, (g + 1) * H)
                    if xlabT is None:
                        nc.tensor.matmul(gps[:, gs], wihT[:, gs], xsT,
                                         start=False, stop=True)
                    else:
                        nc.tensor.matmul(gps[:, gs], wihT[:, gs], xsT,
                                         start=False, stop=False)
                        nc.tensor.matmul(gps[:, gs], wlabT[:, gs], xlabT,
                                         start=False, stop=True)
                return gps

            def lstm_update(pfx, t, gps, cT, hdt, post_c2=None):
                """All four gates share one merged tanh(0.5*.) activation:
                the g gate's weights/bias are pre-scaled 2x host-side so
                tanh(0.5*g') == tanh(g). Bias was accumulated into gps by
                the gate matmuls."""
                Ab = psm.tile([H, 4 * BL], FP, name=f"{pfx}ab{t}", tag="actA")
                nc.scalar.activation(Ab[:, :], gps[:, 0:4 * H], AF.Tanh,
                                     scale=0.5)
                Bv, A, D, C = (Ab[:, 0:BL], Ab[:, BL:2 * BL],
                               Ab[:, 2 * BL:3 * BL], Ab[:, 3 * BL:4 * BL])
                u = psm.tile([H, BL], FP, name=f"{pfx}u{t}", tag="lu")
                nc.vector.scalar_tensor_tensor(u[:, :], A, 1.0,
                                               cT[:, :], op0=OP.add,
                                               op1=OP.mult)
                v = psm.tile([H, BL], FP, name=f"{pfx}v{t}", tag="lv")
                nc.vector.scalar_tensor_tensor(v[:, :], Bv, 1.0,
                                               C[:, :], op0=OP.add,
                                               op1=OP.mult)
                cT2 = pst.tile([H, BL], FP, name=f"{pfx}c{t}", tag=pfx + "c",
                               bufs=2)
                nc.vector.scalar_tensor_tensor(cT2[:, :], u[:, :], 0.5,
                                               v[:, :], op0=OP.mult,
                                               op1=OP.add)
                if post_c2 is not None:
                    post_c2(cT2)
                Tc = psm.tile([H, BL], FP, name=f"{pfx}T{t}", tag="lTc")
                nc.scalar.activation(Tc[:, :], cT2[:, :], AF.Tanh, scale=0.5)
                hT2 = pst.tile([H, BL], hdt, name=f"{pfx}h{t}",
                               tag=pfx + "h", bufs=2)
                nc.vector.scalar_tensor_tensor(hT2[:, :], D, 1.0,
                                               Tc[:, :], op0=OP.add,
                                               op1=OP.mult)
                return hT2, cT2

            def softmax(pfx, t, score, F, scale=1.0):
                """exp-normalize (no max shift); returns bf16 sm [BL,F].
                scale folds the deferred ve[last] factor of the Horner scan
                into the exp (softmax is shift-invariant, so the dropped
                Ve bias needs no fixup)."""
                exps = psm.tile([BL, F], FP, name=f"{pfx}e{t}", tag="exps")
                sume = psm.tile([BL, 1], FP, name=f"{pfx}se{t}", tag="sume")
                nc.scalar.activation(exps[:, :], score[:, :], AF.Exp,
                                     scale=scale, accum_out=sume[:, :])
                rs = psm.tile([BL, 1], FP, name=f"{pfx}rs{t}", tag="rs")
                nc.vector.reciprocal(rs[:, :], sume[:, :])
                sm = psm.tile([BL, F], BF, name=f"{pfx}sm{t}", tag="sm")
                nc.vector.tensor_scalar_mul(sm[:, :], exps[:, :], rs[:, :])
                return sm

            def scan_score(eng, scano, rfull, etan, f0, fn):
                """Horner scan over the (permuted) tau axis: fuses the
                ve-weighted reduction; segment resets ride r[0] == 0."""
                eng.tensor_tensor_scan(
                    scano[:, f0 * T:(f0 + fn) * T],
                    rfull[:, f0 * T:(f0 + fn) * T],
                    etan[:, :], 0.0, op0=OP.mult, op1=OP.add)

            def sum_inner(eng, pool, pfx, t, ci, t2, fn, score_sl, S=T):
                """sum over innermost S of bf16 [BL, fn, S] into score_sl.
                GPSIMD cannot reduce along free dims, so Pool chunks tree
                down and hand the last small reduce to DVE."""
                if not USE_TREE:
                    nc.vector.reduce_sum(
                        score_sl,
                        t2[:, :].rearrange("p (f s) -> p f s", s=S),
                        axis=AX.X)
                    return
                cur, size = t2, S
                lvls = []
                while size > 6:
                    size //= 2
                    lvls.append(size)
                for lvl, ns in enumerate(lvls):
                    nxt = pool.tile([BL, fn * ns], BF,
                                    name=f"{pfx}r{t}_{ci}_{lvl}",
                                    tag=f"{pfx}tr{ci}_{lvl}", bufs=1)
                    cv = cur[:, :].rearrange("p (f s) -> p f s", s=ns * 2)
                    eng.tensor_add(
                        nxt[:, :].rearrange("p (f s) -> p f s", s=ns),
                        cv[:, :, 0:ns], cv[:, :, ns:2 * ns])
                    cur = nxt
                nc.vector.reduce_sum(
                    score_sl,
                    cur[:, :].rearrange("p (f s) -> p f s", s=lvls[-1]),
                    axis=AX.X)

            for _rep in range(reps):
                # =================== STAGE 1 ===================
                mid_sb = pio.tile([BL, T * F2], BF, name="mid_sb",
                                  tag="mid_sb")
                fin_sb = pio.tile([BL, H * T], BF, name="fin_sb",
                                  tag="fin_sb")   # [b, (h, t)] h-major
                out_sb = pio.tile([BL, ND], FP, name="out_sb", tag="out_sb")
                pre2 = pio.tile([BL, F2 * T], BF, name="pre2", tag="pre2")
                midT = pio.tile([H, T * BL], BF, name="midT", tag="midT")
                udT = pio.tile([H, T * BL], BF, name="udT", tag="udT")
                with tc.tile_pool(name="ps1", bufs=1) as ps1:
                    xT_sb = ps1.tile([F1, T * BL], BF, name="xT_sb",
                                     tag="xT_sb")
                    nc.gpsimd.dma_start(out=xT_sb[:, :], in_=d["xT"][:, :])
                    pre1 = ps1.tile([BL, F1 * T], BF, name="pre1",
                                    tag="pre1")

                    with tc.tile_pool(name="pxtt", bufs=1) as pxtt:
                        xTT_sb = pxtt.tile([T + 1, F1 * BL], BF,
                                           name="xTT_sb", tag="xTT_sb")
                        nc.gpsimd.dma_start(out=xTT_sb[:, :],
                                            in_=d["xTT"][:, :])
                        for f0 in range(0, F1, 4):
                            fn = min(4, F1 - f0)
                            pp = ppsum.tile([BL, 4 * T], FP,
                                            name=f"pp1_{f0}", tag="mm_pre")
                            for j in range(fn):
                                nc.tensor.matmul(
                                    pp[:, j * T:(j + 1) * T],
                                    xTT_sb[:, (f0 + j) * BL:(f0 + j + 1) * BL],
                                    w["ue1Ta"][:, :], start=True, stop=True)
                            nc.vector.tensor_copy(
                                pre1[:, f0 * T:(f0 + fn) * T],
                                pp[:, 0:fn * T])

                    nc.vector.tensor_copy(
                        mid_sb[:, :].rearrange("p (t f) -> p t f",
                                               f=F2)[:, :, H],
                        w["lab"][:, :])
                    hT, cT = new_state("s1", BF)
                    weps = ppsum.tile([BL, T], FP, name="we1_0",
                                      tag="mm_small")
                    nc.tensor.matmul(weps[:, :], cT[:, :],
                                     w["we1T_c"][:, :], start=True,
                                     stop=False)
                    nc.tensor.matmul(weps[:, :], hT[:, :],
                                     w["we1T_h"][:, :], start=False,
                                     stop=True)
                    gps = lstm_gates_h("s1", 0, hT, w["whh1T"],
                                       biasrow=w["bias1r"])
                    for t in range(T):
                        we_sb = psm.tile([BL, T], BF, name=f"wes1_{t}",
                                         tag="we_sb")
                        nc.vector.tensor_copy(we_sb[:, :], weps[:, :])
                        score = psm.tile([BL, F1], FP, name=f"sc1_{t}",
                                         tag="sc")
                        scano1 = ps1.tile([BL, F1 * T], BF,
                                          name=f"sco1_{t}", tag="scano1",
                                          bufs=2)
                        s1ch = [(0, 6), (6, 6), (12, 5)]
                        s1eargs = []
                        for ci, (f0, fn) in enumerate(s1ch):
                            earg = ps1.tile([BL, fn * T], BF,
                                            name=f"ea1_{t}_{ci}",
                                            tag=f"big1A{ci}", bufs=2)
                            nc.vector.tensor_add(
                                earg[:, :].rearrange("p (f s) -> p f s",
                                                     s=T),
                                pre1[:, f0 * T:(f0 + fn) * T].rearrange(
                                    "p (f s) -> p f s", s=T),
                                we_sb[:, :].unsqueeze(1).broadcast_to(
                                    [BL, fn, T]))
                            s1eargs.append(earg)
                        for ci, (f0, fn) in enumerate(s1ch):
                            etan = ps1.tile([BL, fn * T], BF,
                                            name=f"et1_{t}_{ci}",
                                            tag=f"big1B{ci}", bufs=2)
                            nc.scalar.activation(etan[:, :],
                                                 s1eargs[ci][:, :], AF.Tanh)
                            scan_score(nc.vector, scano1, w["r1full"],
                                       etan, f0, fn)
                        nc.vector.tensor_copy(
                            score[:, :],
                            scano1[:, :].rearrange("p (f s) -> p f s",
                                                   s=T)[:, :, T - 1])
                        sm = softmax("s1", t, score, F1,
                                     scale=w["vs1"][:, 0:1])
                        smT = ptr.tile([F1, BL], BF, name=f"smT1_{t}",
                                       tag="tr")
                        nc.tensor.transpose(smT[:, :], sm[:, :], idb[:, :])
                        xsT = psm.tile([F1, BL], BF, name=f"xsT1_{t}",
                                       tag="xsT")
                        nc.vector.tensor_mul(xsT[:, :],
                                             xT_sb[:,
                                                   t * BL:(t + 1) * BL],
                                             smT[:, :])
                        lstm_gates_x(gps, xsT, w["wih1T"])
                        wps_n = {}

                        def post_c2_s1(cT2, t=t, wps_n=wps_n):
                            if t + 1 >= T:
                                return
                            wn = ppsum.tile([BL, T], FP,
                                            name=f"we1_{t + 1}",
                                            tag="mm_small")
                            nc.tensor.matmul(wn[:, :], cT2[:, :],
                                             w["we1T_c"][:, :], start=True,
                                             stop=False)
                            wps_n["w"] = wn

                        hT, cT = lstm_update("s1", t, gps, cT, BF,
                                             post_c2=post_c2_s1)
                        if t + 1 < T:
                            weps = wps_n["w"]
                            nc.tensor.matmul(weps[:, :], hT[:, :],
                                             w["we1T_h"][:, :], start=False,
                                             stop=True)
                            gps = lstm_gates_h("s1", t + 1, hT, w["whh1T"],
                                               biasrow=w["bias1r"])
                        nc.gpsimd.tensor_scalar_mul(
                            midT[:, t * BL:(t + 1) * BL], hT[:, :], 0.5)
                        hbt = ptr.tile([BL, H], BF, name=f"hbt_{t}",
                                       tag="tr")
                        nc.tensor.transpose(hbt[:, :], hT[:, :], idb[:, :])
                        nc.vector.tensor_scalar_mul(
                            mid_sb[:, t * F2:t * F2 + H], hbt[:, :], 0.5)

                    # ---- pre2 build: waves of 8 f ----
                    mid3 = mid_sb[:, :].rearrange("p (t f) -> p t f", f=F2)
                    mscs = []
                    for k in range(4):
                        m = ps1.tile([T + 1, BL], BF, name=f"msc{k}",
                                     tag=f"msc{k}")
                        nc.vector.memset(m[32:T + 1, :], 1.0)
                        mscs.append(m)
                    cp_eng = [nc.vector, nc.scalar]
                    for f0 in range(0, F2, 8):
                        fn = min(8, F2 - f0)
                        pp = ppsum.tile([BL, 8 * T], FP, name=f"pp2_{f0}",
                                        tag="mm_pre")
                        mtps = []
                        for j in range(fn):
                            mtp = ptr.tile([T, BL], BF,
                                           name=f"mtp_{f0 + j}", tag="tr")
                            nc.tensor.transpose(mtp[:, :],
                                                mid3[:, :, f0 + j],
                                                idb[:, :])
                            mtps.append(mtp)
                        for j in range(fn):
                            eng = cp_eng[(f0 + j) % 2]
                            msc = mscs[(f0 + j) % 4]
                            if eng is nc.scalar:
                                eng.copy(msc[0:T, :], mtps[j][:, :])
                            else:
                                eng.tensor_copy(msc[0:T, :], mtps[j][:, :])
                            nc.tensor.matmul(pp[:, j * T:(j + 1) * T],
                                             msc[:, :], w["ue2Ta"][:, :],
                                             start=True, stop=True)
                        nc.vector.tensor_copy(pre2[:, f0 * T:(f0 + fn) * T],
                                              pp[:, 0:fn * T])

                # =================== STAGE 2 ===================
                with tc.tile_pool(name="ps2", bufs=1) as ps2:
                    hT, cT = new_state("s2", BF)
                    weps = ppsum.tile([BL, T], FP, name="we2_0",
                                      tag="mm_small")
                    nc.tensor.matmul(weps[:, :], cT[:, :],
                                     w["we2T_c"][:, :], start=True,
                                     stop=False)
                    nc.tensor.matmul(weps[:, :], hT[:, :],
                                     w["we2T_h"][:, :], start=False,
                                     stop=True)
                    gps = lstm_gates_h("s2", 0, hT, w["whh2T"],
                                       biasrow=w["bias2r"])
                    for t in range(T):
                        we_sb = psm.tile([BL, T], BF, name=f"wes2_{t}",
                                         tag="we_sb")
                        nc.vector.tensor_copy(we_sb[:, :], weps[:, :])
                        score = psm.tile([BL, F2], FP, name=f"sc2_{t}",
                                         tag="sc")
                        scano2 = ps2.tile([BL, F2 * T], BF,
                                          name=f"sco2_{t}", tag="scano2",
                                          bufs=2)  # scans fill [lo,hi) only
                        ENG = {"P": nc.gpsimd, "V": nc.vector}
                        eargs = []
                        for ci, (f0, fn, eak, rsk) in enumerate(s2_ch):
                            earg = ps2.tile([BL, fn * T], BF,
                                            name=f"ea2_{t}_{ci}",
                                            tag=f"big2A{ci}", bufs=2)
                            ENG[eak].tensor_add(
                                earg[:, :].rearrange("p (f s) -> p f s",
                                                     s=T),
                                pre2[:, f0 * T:(f0 + fn) * T].rearrange(
                                    "p (f s) -> p f s", s=T),
                                we_sb[:, :].unsqueeze(1).broadcast_to(
                                    [BL, fn, T]))
                            eargs.append(earg)
                        etans = []
                        for ci, (f0, fn, eak, rsk) in enumerate(s2_ch):
                            etan = ps2.tile([BL, fn * T], BF,
                                            name=f"et2_{t}_{ci}",
                                            tag=f"big2B{ci}", bufs=2)
                            nc.scalar.activation(etan[:, :],
                                                 eargs[ci][:, :], AF.Tanh)
                            etans.append(etan)
                            if rsk == "V":
                                scan_score(nc.vector, scano2, w["r2full"],
                                           etan, f0, fn)
                            else:
                                t2 = ps2.tile([BL, fn * T], BF,
                                              name=f"t22_{t}_{ci}",
                                              tag=f"big2A{ci}", bufs=2)
                                nc.gpsimd.tensor_mul(
                                    t2[:, :].rearrange("p (f s) -> p f s",
                                                       s=T),
                                    etan[:, :].rearrange(
                                        "p (f s) -> p f s", s=T),
                                    w["ve2_rep"][:, :].unsqueeze(1)
                                    .broadcast_to([BL, fn, T]))
                                sum_inner(nc.gpsimd, ps2, "s2", t, ci, t2,
                                          fn, score[:, f0:f0 + fn])
                        lo, hi = S2_SCAN_LO, S2_SCAN_HI
                        nc.vector.tensor_copy(
                            score[:, lo:hi],
                            scano2[:, lo * T:hi * T].rearrange(
                                "p (f s) -> p f s", s=T)[:, :, T - 1])
                        sm = softmax("s2", t, score, F2,
                                     scale=w["vs2"][:, 0:1])
                        smT = ptr.tile([H, BL], BF, name=f"smT2_{t}",
                                       tag="tr")
                        nc.tensor.transpose(smT[:, :], sm[:, 0:H],
                                            idb[:, :])
                        xsT = psm.tile([H, BL], BF, name=f"xsT2_{t}",
                                       tag="xsT")
                        nc.vector.tensor_mul(xsT[:, :],
                                             midT[:, t * BL:(t + 1) * BL],
                                             smT[:, :])
                        xlab = psm.tile([BL, 1], FP, name=f"xl_{t}",
                                        tag="s2xl")
                        nc.vector.tensor_mul(xlab[:, :],
                                             w["lab"][:, t:t + 1],
                                             sm[:, H:H + 1])
                        xlT_ps = ptr.tile([1, BL], FP, name=f"xlT_{t}",
                                          tag="tr")
                        nc.tensor.transpose(xlT_ps[:, :], xlab[:, :],
                                            idf[:, :])
                        xlabT = psm.tile([1, BL], BF, name=f"xls_{t}",
                                         tag="s2xls")
                        nc.scalar.copy(xlabT[:, :], xlT_ps[:, :])
                        lstm_gates_x(gps, xsT, w["wih2T_h"], xlabT=xlabT,
                                     wlabT=w["wih2T_l"])
                        wps_n = {}

                        def post_c2_s2(cT2, t=t, wps_n=wps_n):
                            if t + 1 >= T:
                                return
                            wn = ppsum.tile([BL, T], FP,
                                            name=f"we2_{t + 1}",
                                            tag="mm_small")
                            nc.tensor.matmul(wn[:, :], cT2[:, :],
                                             w["we2T_c"][:, :], start=True,
                                             stop=False)
                            wps_n["w"] = wn

                        hT, cT = lstm_update("s2", t, gps, cT, BF,
                                             post_c2=post_c2_s2)
                        if t + 1 < T:
                            weps = wps_n["w"]
                            nc.tensor.matmul(weps[:, :], hT[:, :],
                                             w["we2T_h"][:, :], start=False,
                                             stop=True)
                            gps = lstm_gates_h("s2", t + 1, hT, w["whh2T"],
                                               biasrow=w["bias2r"])
                        hbt = ptr.tile([BL, H], BF, name=f"hbt2_{t}",
                                       tag="tr")
                        nc.tensor.transpose(hbt[:, :], hT[:, :], idb[:, :])
                        nc.scalar.mul(
                            fin_sb[:, :].rearrange("p (h t) -> p h t",
                                                   t=T)[:, :, t],
                            hbt[:, :], 0.5)
                        up = ppsum.tile([H, BL], FP, name=f"udp_{t}",
                                        tag="mm_small")
                        nc.tensor.matmul(up[:, :], w["udT"][:, :], hT[:, :],
                                         start=True, stop=True)
                        nc.scalar.activation(udT[:, t * BL:(t + 1) * BL],
                                             up[:, :], AF.Identity,
                                             bias=w["udbT"][:, 0:1])

                # =================== DECODER ===================
                # Attention computed transposed [h, (t, b)]: the score
                # contraction over h=128 then runs on the PE (48 tiny
                # matmuls with the constant vd stationary) instead of a
                # DVE multiply + tree, and softmax normalization is
                # deferred into a per-partition scale of din.
                with tc.tile_pool(name="pdec", bufs=1) as pdec:
                    fin3 = fin_sb[:, :].rearrange("p (h s) -> p h s", s=T)
                    hT, cT = new_state("sd", BF)
                    hTb = hT
                    wdT_ps = ppsum.tile([H, BL], FP, name="wd_0",
                                        tag="mm_small")
                    nc.tensor.matmul(wdT_ps[:, :], w["wdT_c"][:, :],
                                     cT[:, :], start=True, stop=False)
                    nc.tensor.matmul(wdT_ps[:, :], w["wdT_h"][:, :],
                                     hTb[:, :], start=False, stop=True)
                    gps = lstm_gates_h("sd", 0, hTb, w["whhdT"],
                                       biasrow=w["biasd"])
                    for t in range(ND):
                        wdTs = psm.tile([H, BL], BF, name=f"wds_{t}",
                                        tag="we_sb")
                        nc.vector.tensor_copy(wdTs[:, :], wdT_ps[:, :])
                        achunks = ([(t0, tn, nc.vector) for t0, tn in TD_CH]
                                   + [(*TD_POOL, nc.gpsimd)])
                        eargs = []
                        for ci, (t0, tn, eng) in enumerate(achunks):
                            argT = pdec.tile([H, tn * BL], BF,
                                             name=f"ead_{t}_{ci}",
                                             tag=f"bigdA{ci}", bufs=2)
                            eng.tensor_add(
                                argT[:, :].rearrange("p (s b) -> p s b",
                                                     b=BL),
                                udT[:, t0 * BL:(t0 + tn) * BL].rearrange(
                                    "p (s b) -> p s b", b=BL),
                                wdTs[:, :].unsqueeze(1).broadcast_to(
                                    [H, tn, BL]))
                            eargs.append(argT)
                        scps = ppsum.tile([BL, T], FP, name=f"scT_{t}",
                                          tag="mm_small")

                        def etan_scores(ci):
                            t0, tn, eng = achunks[ci]
                            etan = pdec.tile([H, tn * BL], BF,
                                             name=f"etd_{t}_{ci}",
                                             tag=f"bigdB{ci}", bufs=2)
                            nc.scalar.activation(etan[:, :],
                                                 eargs[ci][:, :], AF.Tanh)
                            for j in range(tn):
                                nc.tensor.matmul(
                                    scps[:, t0 + j:t0 + j + 1],
                                    etan[:, j * BL:(j + 1) * BL],
                                    w["vdcol"][:, :],
                                    start=True, stop=True)

                        def din_half(ph, t0, tn, dout):
                            expv = psm.tile([BL, tn], BF,
                                            name=f"exb{ph}_{t}",
                                            tag=f"expsb{ph}")
                            Zp = psm.tile([BL, 1], FP, name=f"Z{ph}_{t}",
                                          tag=f"Zs{ph}")
                            nc.scalar.activation(expv[:, :],
                                                 scps[:, t0:t0 + tn],
                                                 AF.Exp,
                                                 accum_out=Zp[:, :])
                            dchunks = ([(h0, hn, nc.vector)
                                        for h0, hn in DIN_CH]
                                       + [(*DIN_POOL, nc.gpsimd)])
                            for ci, (h0, hn, eng) in enumerate(dchunks):
                                dmul = pdec.tile([BL, hn * tn], BF,
                                                 name=f"dm{ph}_{t}_{ci}",
                                                 tag=f"bigd{ph}D{ci}",
                                                 bufs=1)
                                eng.tensor_mul(
                                    dmul[:, :].rearrange(
                                        "p (h s) -> p h s", s=tn),
                                    fin3[:, h0:h0 + hn, t0:t0 + tn],
                                    expv[:, :].unsqueeze(1).broadcast_to(
                                        [BL, hn, tn]))
                                sum_inner(eng, pdec, f"sd{ph}", t, ci,
                                          dmul, hn, dout[:, h0:h0 + hn],
                                          S=tn)
                            return Zp

                        etan_scores(0)
                        etan_scores(1)
                        dinA = psm.tile([BL, H], FP, name=f"dinA_{t}",
                                        tag="sddinA")
                        ZA = din_half("A", 0, TD2, dinA)
                        etan_scores(2)
                        etan_scores(3)
                        dinB = psm.tile([BL, H], FP, name=f"dinB_{t}",
                                        tag="sddinB")
                        ZB = din_half("B", TD2, T - TD2, dinB)
                        Zs = psm.tile([BL, 1], FP, name=f"Zs_{t}", tag="Zs")
                        nc.vector.tensor_add(Zs[:, :], ZA[:, :], ZB[:, :])
                        rsd = psm.tile([BL, 1], FP, name=f"rsd_{t}",
                                       tag="rs")
                        nc.vector.reciprocal(rsd[:, :], Zs[:, :])
                        din = psm.tile([BL, H], FP, name=f"din_{t}",
                                       tag="sddin")
                        nc.vector.tensor_add(din[:, :], dinA[:, :],
                                             dinB[:, :])
                        dinn = psm.tile([BL, H], FP, name=f"dinn_{t}",
                                        tag="sddinN")
                        nc.vector.tensor_scalar_mul(dinn[:, :], din[:, :],
                                                    rsd[:, :])
                        dinT_ps = ptr.tile([H, BL], FP, name=f"dTp_{t}",
                                           tag="tr")
                        nc.tensor.transpose(dinT_ps[:, :], dinn[:, :],
                                            idf[:, :])
                        dinT = psm.tile([H, BL], BF, name=f"dT_{t}",
                                        tag="sddinT")
                        nc.vector.tensor_copy(dinT[:, :], dinT_ps[:, :])
                        lstm_gates_x(gps, dinT, w["wihdT"])
                        wps_n = {}

                        def post_c2_sd(cT2, t=t, wps_n=wps_n):
                            if t + 1 >= ND:
                                return
                            wn = ppsum.tile([H, BL], FP,
                                            name=f"wd_{t + 1}",
                                            tag="mm_small")
                            nc.tensor.matmul(wn[:, :], w["wdT_c"][:, :],
                                             cT2[:, :], start=True,
                                             stop=False)
                            wps_n["w"] = wn

                        hT, cT = lstm_update("sd", t, gps, cT, BF,
                                             post_c2=post_c2_sd)
                        hTb = hT
                        if t + 1 < ND:
                            wdT_ps = wps_n["w"]
                            nc.tensor.matmul(wdT_ps[:, :], w["wdT_h"][:, :],
                                             hTb[:, :], start=False,
                                             stop=True)
                            gps = lstm_gates_h("sd", t + 1, hTb,
                                               w["whhdT"],
                                               biasrow=w["biasd"])
                        op = ppsum.tile([BL, 1], FP, name=f"op_{t}",
                                        tag="mm_small")
                        nc.tensor.matmul(op[:, :], hT[:, :],
                                         w["regWT"][:, :], start=True,
                                         stop=True)
                        nc.vector.tensor_copy(out_sb[:, t:t + 1], op[:, :])

                    outf = pdec.tile([BL, TD], FP, name="outf", tag="outf")
                    nc.vector.tensor_scalar_add(outf[:, :], out_sb[:, 6:ND],
                                                w["regb_rep"][:, :])
                    nc.gpsimd.dma_start(out=out_d[:, :], in_=outf[:, :])

    if split_waits:
        _split_matmul_waits(nc)
    return nc


# ---------------- host-side prep ----------------
def _prep_weights(i):
    f32 = lambda a: np.ascontiguousarray(a, np.float32)
    try:
        import ml_dtypes
        bf16 = lambda a: np.ascontiguousarray(
            np.asarray(a, np.float32).astype(ml_dtypes.bfloat16))
    except ImportError:
        import jax.numpy as jnp
        bf16 = lambda a: np.ascontiguousarray(
            np.asarray(jnp.asarray(a, jnp.bfloat16)))
    # gate order [i, f, o, g]: all four gates share the tanh(0.5*(.))
    # activation; the g gate's weights and bias are pre-scaled 2x so
    # tanh(0.5*(2g)) == tanh(g).
    gperm = np.concatenate([np.arange(0, H), np.arange(H, 2 * H),
                            np.arange(3 * H, 4 * H), np.arange(2 * H, 3 * H)])
    gscale = np.concatenate([np.ones(3 * H, np.float32),
                             np.full(H, 2.0, np.float32)])

    def bias_hilo(b):
        bp = np.asarray(b, np.float32)[gperm] * gscale
        hi = np.asarray(bf16(bp), np.float32)
        return bp, hi, bp - hi

    def horner(ve):
        """Ascending-|ve| tau permutation + Horner scan ratios (r[0]=0
        doubles as the per-feature segment reset)."""
        perm = np.argsort(np.abs(np.asarray(ve, np.float64)))
        vp = np.asarray(ve, np.float64)[perm]
        r = np.zeros(T, np.float64)
        r[1:] = vp[:-1] / vp[1:]
        return perm, r, np.float32(vp[-1])

    perm1, r1, vs1 = horner(i["Ve1_W"][0])
    perm2, r2, vs2 = horner(i["Ve2_W"][0])
    w = {}
    w["we1T_h"] = bf16(0.5 * i["We1_W"][:, :H].T[:, perm1])
    w["we1T_c"] = f32(0.5 * i["We1_W"][:, H:].T[:, perm1])
    w["ue1Ta"] = bf16(np.concatenate(
        [i["Ue1_W"].T, i["Ue1_b"][None, :]], 0)[:, perm1])
    w["r1full"] = f32(np.tile(np.tile(r1, F1)[None, :], (BL, 1)))
    w["vs1"] = f32(np.full((BL, 1), vs1))
    b1, b1hi, b1lo = bias_hilo(i["e1_bih"] + i["e1_bhh"])
    w["wih1T"] = bf16(i["e1_Wih"].T[:, gperm] * gscale)
    w["bias1r"] = bf16(np.stack([b1hi, b1lo], 0))
    w["whh1T"] = bf16(0.5 * i["e1_Whh"].T[:, gperm] * gscale)
    w["we2T_h"] = bf16(0.5 * i["We2_W"][:, :H].T[:, perm2])
    w["we2T_c"] = f32(0.5 * i["We2_W"][:, H:].T[:, perm2])
    w["ue2Ta"] = bf16(np.concatenate(
        [i["Ue2_W"].T, i["Ue2_b"][None, :]], 0)[:, perm2])
    w["r2full"] = f32(np.tile(np.tile(r2, F2)[None, :], (BL, 1)))
    w["ve2_rep"] = bf16(np.tile(i["Ve2_W"][0][perm2][None, :], (BL, 1)))
    w["vs2"] = f32(np.full((BL, 1), vs2))
    b2, b2hi, b2lo = bias_hilo(i["e2_bih"] + i["e2_bhh"])
    w["wih2T_h"] = bf16(i["e2_Wih"][:, :H].T[:, gperm] * gscale)
    w["wih2T_l"] = bf16(i["e2_Wih"][:, H:].T[:, gperm] * gscale)
    w["bias2r"] = bf16(np.stack([b2hi, b2lo], 0))
    w["whh2T"] = bf16(0.5 * i["e2_Whh"].T[:, gperm] * gscale)
    w["udT"] = bf16(0.5 * i["Ud_W"].T)
    w["udbT"] = f32(i["Ud_b"][:, None])
    w["wdT_h"] = bf16(0.5 * i["Wd_W"][:, :H].T)
    w["wdT_c"] = f32(0.5 * i["Wd_W"][:, H:].T)
    w["vdcol"] = bf16(i["Vd_W"][0][:, None])
    w["wihdT"] = bf16(i["d_Wih"].T[:, gperm] * gscale)
    w["whhdT"] = bf16(0.5 * i["d_Whh"].T[:, gperm] * gscale)
    bd, _, _ = bias_hilo(i["d_bih"] + i["d_bhh"])
    _hi = np.asarray(bf16(bd), np.float32)
    w["biasd"] = np.concatenate([bf16(bd)[None, :], bf16(bd - _hi)[None, :]],
                                0)
    w["ones2"] = bf16(np.ones((2, BL)))
    w["regWT"] = bf16(0.5 * i["reg_W"].T)
    w["regb_rep"] = f32(np.tile(i["reg_b"][None, :], (BL, 1)))
    w["identf"] = np.eye(128, dtype=np.float32)
    w["identb"] = bf16(np.eye(128))
    w["_bf16"] = bf16
    return w


def prep_in_maps(inputs):
    i = {k: np.asarray(v) for k, v in inputs.items()}
    wshared = _prep_weights(i)
    bf16 = wshared.pop("_bf16")
    maps = []
    for c in range(NCORES):
        sl = slice(c * BL, (c + 1) * BL)
        x = np.ascontiguousarray(i["input_p_q"][sl][:, :, 1:], np.float32)
        lab = np.ascontiguousarray(i["label_p"][sl], np.float32)
        m = dict(wshared)
        xTT = x.transpose(1, 2, 0).reshape(T, F1 * BL)
        m["xTT"] = bf16(np.concatenate(
            [xTT, np.ones((1, F1 * BL), np.float32)], 0))
        m["xT"] = bf16(x.transpose(2, 1, 0).reshape(F1, T * BL))
        m["lab"] = lab
        maps.append(m)
    return maps


_CACHE = {}


def kernel(**inputs):
    from concourse.bass_utils import run_bass_kernel_spmd
    if "nc" not in _CACHE:
        _CACHE["nc"] = build_bass()
    nc = _CACHE["nc"]
    in_maps = prep_in_maps(inputs)
    res = run_bass_kernel_spmd(nc, in_maps, list(range(NCORES)))
    out = np.concatenate([res.results[c]["out"] for c in range(NCORES)], 0)
    return out.astype(np.float32)


if __name__ == "__main__":
    nc = build_bass()
    print("built ok")

